# revision 5
# baseline (speedup 1.0000x reference)
"""Trainium2 Bass kernel for BERT post-training baseline loss
(two CRF tagging heads + sentiment head over [64, 512, 1024] hidden states).

Strategy: data-parallel over batch across 8 NeuronCores (8 sequences/core).
Per core everything is local (no collectives):
  - hidden shard streams in natural [token, h] layout (tokens on partitions)
  - pooling (per-batch token sums) via ones-vector matmul on PE (contracts tokens)
  - per-token logits need the h-contraction, so each [128,128] block is
    PE-transposed (f32r) into PSUM (4 blocks packed per bank), copied to SBUF,
    then W[128h,6].T @ X^T[128h,512tok] accumulates logits^T [6,512] per batch
  - CRF emission scores via one tensor_tensor_reduce per batch against a
    host-built masked one-hot [6,512]
Host epilogue: bias adds, sentiment head, label-only transition scores and
mask counts, final scalar loss. All heavy (hidden-dependent) math is on-device.
"""
import sys

sys.path.insert(0, "/opt/trn_rl_repo")

import numpy as np

import concourse.bacc as bacc
import concourse.tile as tile
from concourse import mybir
from concourse import bass_utils

F32 = mybir.dt.float32
F32R = mybir.dt.float32r

B, S, H, T = 64, 512, 1024, 3
NCORES = 8
BPC = B // NCORES           # batches per core = 8
TOK = BPC * S               # tokens per core = 4096
NHC = H // 128              # h-chunks = 8
NTT = S // 128              # token tiles per batch = 4


def _build_nc():
    nc = bacc.Bacc("TRN2", target_bir_lowering=False, debug=False)

    x_d = nc.dram_tensor("x", [TOK, H], F32R, kind="ExternalInput")
    wc_d = nc.dram_tensor("wc", [128, NHC * 6], F32R, kind="ExternalInput")
    oh_d = nc.dram_tensor("oh", [6, TOK], F32, kind="ExternalInput")
    id_d = nc.dram_tensor("ident", [128, 128], F32R, kind="ExternalInput")
    ones_d = nc.dram_tensor("ones", [128, 1], F32R, kind="ExternalInput")

    lg_d = nc.dram_tensor("logits_t", [6, TOK], F32, kind="ExternalOutput")
    pl_d = nc.dram_tensor("pooled", [1, BPC * H], F32, kind="ExternalOutput")
    em_d = nc.dram_tensor("emit", [6, BPC], F32, kind="ExternalOutput")

    x_v = x_d.ap().rearrange("(b i p) h -> b p i h", b=BPC, i=NTT, p=128)

    with tile.TileContext(nc) as tc:
        with tc.tile_pool(name="const", bufs=1) as cp, \
             tc.tile_pool(name="xin", bufs=3) as xp, \
             tc.tile_pool(name="xt", bufs=10) as xtp, \
             tc.tile_pool(name="small", bufs=3) as smp, \
             tc.tile_pool(name="xtps", bufs=3, space="PSUM") as xtps, \
             tc.tile_pool(name="poolps", bufs=1, space="PSUM") as plps, \
             tc.tile_pool(name="logps", bufs=2, space="PSUM") as lgps:

            wc_sb = cp.tile([128, NHC * 6], F32R, tag="wc")
            oh_sb = cp.tile([6, TOK], F32, tag="oh")
            id_sb = cp.tile([128, 128], F32R, tag="id")
            on_sb = cp.tile([128, 1], F32R, tag="ones")
            pl_sb = cp.tile([1, BPC * H], F32, tag="pooled")
            em_sb = cp.tile([6, BPC], F32, tag="emit")
            nc.sync.dma_start(wc_sb[:], wc_d.ap())
            nc.sync.dma_start(oh_sb[:], oh_d.ap())
            nc.sync.dma_start(id_sb[:], id_d.ap())
            nc.sync.dma_start(on_sb[:], ones_d.ap())

            for b in range(BPC):
                xb = xp.tile([128, NTT * H], F32R, tag="xb")
                nc.sync.dma_start(
                    xb[:].rearrange("p (i h) -> p i h", i=NTT), x_v[b]
                )

                # pooling: per-batch token sums, [1, 512] psum per h-half
                for half in range(2):
                    pps = plps.tile([1, 512], F32, tag=f"pl{half}")
                    for i in range(NTT):
                        nc.tensor.matmul(
                            pps[:],
                            on_sb[:],
                            xb[:, i * H + half * 512 : i * H + half * 512 + 512],
                            start=(i == 0),
                            stop=(i == NTT - 1),
                        )
                    nc.any.tensor_copy(
                        pl_sb[0:1, b * H + half * 512 : b * H + half * 512 + 512],
                        pps[:],
                    )

                # transpose all 32 [128,128] blocks; pack 4 (token tiles) per
                # PSUM bank as [128h, 512tok]; evacuate to SBUF for the
                # logits matmul
                xts = []
                for hc in range(NHC):
                    tps = xtps.tile([128, 512], F32R, tag="xtps")
                    for i in range(NTT):
                        nc.tensor.transpose(
                            tps[:, i * 128 : (i + 1) * 128],
                            xb[:, i * H + hc * 128 : i * H + (hc + 1) * 128],
                            id_sb[:],
                        )
                    xt = xtp.tile([128, 512], F32R, tag="xt")
                    nc.any.tensor_copy(xt[:], tps[:])
                    xts.append(xt)

                # logits^T [6, 512] for this batch
                lps = lgps.tile([6, 512], F32, tag="lg")
                for hc in range(NHC):
                    nc.tensor.matmul(
                        lps[:],
                        wc_sb[:, hc * 6 : (hc + 1) * 6],
                        xts[hc][:],
                        start=(hc == 0),
                        stop=(hc == NHC - 1),
                    )
                lsb = smp.tile([6, 512], F32, tag="lsb")
                nc.any.tensor_copy(lsb[:], lps[:])
                nc.sync.dma_start(lg_d.ap()[:, b * S : (b + 1) * S], lsb[:])

                # CRF emission partial sums: per-class, this batch
                scr = smp.tile([6, 512], F32, tag="scr")
                nc.vector.tensor_mul(scr[:], lsb[:], oh_sb[:, b * S : (b + 1) * S])
                nc.vector.tensor_reduce(
                    em_sb[:, b : b + 1],
                    scr[:],
                    axis=mybir.AxisListType.X,
                    op=mybir.AluOpType.add,
                )

            nc.sync.dma_start(pl_d.ap(), pl_sb[:])
            nc.sync.dma_start(em_d.ap(), em_sb[:])

    nc.compile()
    return nc


_NC_CACHE = None


def _get_nc():
    global _NC_CACHE
    if _NC_CACHE is None:
        _NC_CACHE = _build_nc()
    return _NC_CACHE


def _prep_inputs(inputs):
    hs = np.ascontiguousarray(inputs["hidden_states"], dtype=np.float32)
    mask = inputs["attention_mask"]
    al = inputs["aspect_labels"]
    ol = inputs["opinion_labels"]
    maskf = mask.astype(np.float32)

    # combined head weights -> [128, 8*6]: column block hc holds W[hc*128:(hc+1)*128, :]
    w6 = np.concatenate(
        [np.asarray(inputs["W_aspect"], np.float32),
         np.asarray(inputs["W_opinion"], np.float32)], axis=1
    )  # [1024, 6]
    wc = np.ascontiguousarray(
        w6.reshape(NHC, 128, 6).transpose(1, 0, 2).reshape(128, NHC * 6)
    )

    # masked one-hots [B, S, 6] -> per-core [6, TOK]
    oh = np.zeros((B, S, 6), dtype=np.float32)
    bi = np.arange(B)[:, None]
    si = np.arange(S)[None, :]
    oh[bi, si, al] = maskf
    oh[bi, si, np.asarray(ol) + 3] = maskf

    ident = np.eye(128, dtype=np.float32)
    ones = np.ones((128, 1), dtype=np.float32)

    in_maps = []
    for c in range(NCORES):
        x_c = hs[c * BPC : (c + 1) * BPC].reshape(TOK, H)
        oh_c = np.ascontiguousarray(
            oh[c * BPC : (c + 1) * BPC].reshape(TOK, 6).T
        )
        in_maps.append({"x": x_c, "wc": wc, "oh": oh_c, "ident": ident,
                       "ones": ones})
    return in_maps


def _host_epilogue(inputs, results):
    mask = np.asarray(inputs["attention_mask"])
    al = np.asarray(inputs["aspect_labels"])
    ol = np.asarray(inputs["opinion_labels"])
    b_a = np.asarray(inputs["b_aspect"], np.float32)
    b_o = np.asarray(inputs["b_opinion"], np.float32)
    W_s = np.asarray(inputs["W_sent"], np.float32)
    b_s = np.asarray(inputs["b_sent"], np.float32)
    tr_a = np.asarray(inputs["trans_aspect"], np.float32)
    tr_o = np.asarray(inputs["trans_opinion"], np.float32)
    maskf = mask.astype(np.float32)

    # logits: [6, TOK] per core -> [B, S, 6]
    lt = np.stack([r["logits_t"] for r in results])          # [8, 6, 4096]
    lg = lt.reshape(NCORES, 6, BPC, S).transpose(0, 2, 3, 1).reshape(B, S, 6)
    aspect_logits = lg[..., 0:3] + b_a
    opinion_logits = lg[..., 3:6] + b_o

    # sentiment head from pooled sums
    pooled = np.concatenate(
        [r["pooled"].reshape(BPC, H) for r in results], axis=0
    )  # [64, 1024]
    sentiment_logits = (pooled / np.float32(S)) @ W_s + b_s

    # CRF log-likelihoods
    emit = np.stack([r["emit"] for r in results])             # [8, 6, BPC]
    emit_a = float(emit[:, 0:3, :].sum())
    emit_o = float(emit[:, 3:6, :].sum())
    # bias contribution to emission scores (zero biases in practice)
    cnt_a = np.array([(maskf * (al == c)).sum() for c in range(T)])
    cnt_o = np.array([(maskf * (ol == c)).sum() for c in range(T)])
    emit_a += float(b_a @ cnt_a)
    emit_o += float(b_o @ cnt_o)

    pm = maskf[:, 1:] * maskf[:, :-1]
    trans_a = float((tr_a[al[:, :-1], al[:, 1:]] * pm).sum())
    trans_o = float((tr_o[ol[:, :-1], ol[:, 1:]] * pm).sum())

    cnt = float(maskf.sum())
    loss = -(emit_a + trans_a) / cnt - (emit_o + trans_o) / cnt

    return (
        aspect_logits.astype(np.float32),
        opinion_logits.astype(np.float32),
        sentiment_logits.astype(np.float32),
        np.float32(loss),
    )


def _run(inputs, trace=False):
    nc = _get_nc()
    in_maps = _prep_inputs(inputs)
    res = bass_utils.run_bass_kernel_spmd(
        nc, in_maps, core_ids=list(range(NCORES)), trace=trace
    )
    return _host_epilogue(inputs, res.results), res


def kernel(**inputs):
    out, _ = _run(inputs)
    return out


# revision 6
# speedup vs baseline: 1.0154x; 1.0154x over previous
"""Trainium2 Bass kernel for BERT post-training baseline loss
(two CRF tagging heads + sentiment head over [64, 512, 1024] hidden states).

Strategy: data-parallel over batch across 8 NeuronCores (8 sequences/core).
Per core everything is local (no collectives):
  - hidden shard streams in natural [token, h] layout (tokens on partitions)
  - each [128,128] block is PE-transposed (f32r) into PSUM (4 blocks packed
    per bank), evacuated to SBUF by ScalarE activation-copies whose inline
    accum_out simultaneously produces the per-batch token sums (pooling
    comes for free), then W[128h,6].T @ X^T[128h,512tok] accumulates
    logits^T [6,512] per batch on the PE
  - CRF emission scores via tensor-mul + reduce on VectorE against a
    host-built masked one-hot [6,512]
Host epilogue: bias adds, sentiment head (25K flops), label-only transition
scores and mask counts, final scalar loss. All hidden-dependent math is
on-device.
"""
import sys

sys.path.insert(0, "/opt/trn_rl_repo")

import numpy as np

import concourse.bacc as bacc
import concourse.tile as tile
from concourse import mybir
from concourse import bass_utils

F32 = mybir.dt.float32
F32R = mybir.dt.float32r

B, S, H, T = 64, 512, 1024, 3
NCORES = 8
BPC = B // NCORES           # batches per core = 8
TOK = BPC * S               # tokens per core = 4096
NHC = H // 128              # h-chunks = 8
NTT = S // 128              # token tiles per batch = 4


def _build_nc():
    nc = bacc.Bacc("TRN2", target_bir_lowering=False, debug=False)

    x_d = nc.dram_tensor("x", [TOK, H], F32R, kind="ExternalInput")
    wc_d = nc.dram_tensor("wc", [128, NHC * 6], F32R, kind="ExternalInput")
    oh_d = nc.dram_tensor("oh", [6, TOK], F32, kind="ExternalInput")
    id_d = nc.dram_tensor("ident", [128, 128], F32R, kind="ExternalInput")

    lg_d = nc.dram_tensor("logits_t", [6, TOK], F32, kind="ExternalOutput")
    pl_d = nc.dram_tensor("pooled_t", [128, NHC * BPC], F32, kind="ExternalOutput")
    em_d = nc.dram_tensor("emit", [6, BPC], F32, kind="ExternalOutput")

    # [b, half, p, j, h]: token tile i = 2*half + j within batch b
    x_v = x_d.ap().rearrange(
        "(b half j p) h -> b half p j h", b=BPC, half=2, j=2, p=128
    )

    with tile.TileContext(nc) as tc:
        with tc.tile_pool(name="const", bufs=1) as cp, \
             tc.tile_pool(name="xin", bufs=3) as xp, \
             tc.tile_pool(name="xt", bufs=12) as xtp, \
             tc.tile_pool(name="small", bufs=3) as smp, \
             tc.tile_pool(name="xtps", bufs=5, space="PSUM") as xtps, \
             tc.tile_pool(name="logps", bufs=2, space="PSUM") as lgps:

            wc_sb = cp.tile([128, NHC * 6], F32R, tag="wc")
            oh_sb = cp.tile([6, TOK], F32, tag="oh")
            id_sb = cp.tile([128, 128], F32R, tag="id")
            pl_sb = cp.tile([128, NHC * BPC], F32, tag="pooledt")
            em_sb = cp.tile([6, BPC], F32, tag="emit")
            nc.sync.dma_start(id_sb[:], id_d.ap())
            nc.sync.dma_start(wc_sb[:], wc_d.ap())
            nc.sync.dma_start(oh_sb[:], oh_d.ap())

            for b in range(BPC):
                xb = xp.tile([128, NTT * H], F32R, tag="xb")
                xbv = xb[:].rearrange("p (half j h) -> half p j h", half=2, j=2)
                nc.sync.dma_start(xbv[0], x_v[b, 0])
                nc.sync.dma_start(xbv[1], x_v[b, 1])

                # transpose all 32 [128,128] blocks; pack 4 (token tiles) per
                # PSUM bank as [128h, 512tok]; evacuate via ScalarE copy whose
                # accum_out yields the per-batch token sums (pooling)
                xts = []
                for hc in range(NHC):
                    tps = xtps.tile([128, 512], F32R, tag="xtps")
                    for i in range(NTT):
                        nc.tensor.transpose(
                            tps[:, i * 128 : (i + 1) * 128],
                            xb[:, i * H + hc * 128 : i * H + (hc + 1) * 128],
                            id_sb[:],
                        )
                    xt = xtp.tile([128, 512], F32R, tag="xt")
                    nc.scalar.activation(
                        xt[:],
                        tps[:],
                        mybir.ActivationFunctionType.Copy,
                        accum_out=pl_sb[:, hc * BPC + b : hc * BPC + b + 1],
                    )
                    xts.append(xt)

                # logits^T [6, 512] for this batch
                lps = lgps.tile([6, 512], F32, tag="lg")
                for hc in range(NHC):
                    nc.tensor.matmul(
                        lps[:],
                        wc_sb[:, hc * 6 : (hc + 1) * 6],
                        xts[hc][:],
                        start=(hc == 0),
                        stop=(hc == NHC - 1),
                    )
                lsb = smp.tile([6, 512], F32, tag="lsb")
                nc.vector.tensor_copy(lsb[:], lps[:])
                nc.scalar.dma_start(lg_d.ap()[:, b * S : (b + 1) * S], lsb[:])

                # CRF emission partial sums: per-class, this batch
                scr = smp.tile([6, 512], F32, tag="scr")
                nc.vector.tensor_mul(scr[:], lsb[:], oh_sb[:, b * S : (b + 1) * S])
                nc.vector.tensor_reduce(
                    em_sb[:, b : b + 1],
                    scr[:],
                    axis=mybir.AxisListType.X,
                    op=mybir.AluOpType.add,
                )

            nc.scalar.dma_start(pl_d.ap(), pl_sb[:])
            nc.scalar.dma_start(em_d.ap(), em_sb[:])

    nc.compile()
    return nc


_NC_CACHE = None


def _get_nc():
    global _NC_CACHE
    if _NC_CACHE is None:
        _NC_CACHE = _build_nc()
    return _NC_CACHE


def _prep_inputs(inputs):
    hs = np.ascontiguousarray(inputs["hidden_states"], dtype=np.float32)
    mask = inputs["attention_mask"]
    al = inputs["aspect_labels"]
    ol = inputs["opinion_labels"]
    maskf = mask.astype(np.float32)

    # combined head weights -> [128, 8*6]: column block hc holds W[hc*128:(hc+1)*128, :]
    w6 = np.concatenate(
        [np.asarray(inputs["W_aspect"], np.float32),
         np.asarray(inputs["W_opinion"], np.float32)], axis=1
    )  # [1024, 6]
    wc = np.ascontiguousarray(
        w6.reshape(NHC, 128, 6).transpose(1, 0, 2).reshape(128, NHC * 6)
    )

    # masked one-hots [B, S, 6] -> per-core [6, TOK]
    oh = np.zeros((B, S, 6), dtype=np.float32)
    bi = np.arange(B)[:, None]
    si = np.arange(S)[None, :]
    oh[bi, si, al] = maskf
    oh[bi, si, np.asarray(ol) + 3] = maskf

    ident = np.eye(128, dtype=np.float32)

    in_maps = []
    for c in range(NCORES):
        x_c = hs[c * BPC : (c + 1) * BPC].reshape(TOK, H)
        oh_c = np.ascontiguousarray(
            oh[c * BPC : (c + 1) * BPC].reshape(TOK, 6).T
        )
        in_maps.append({"x": x_c, "wc": wc, "oh": oh_c, "ident": ident})
    return in_maps


def _host_epilogue(inputs, results):
    mask = np.asarray(inputs["attention_mask"])
    al = np.asarray(inputs["aspect_labels"])
    ol = np.asarray(inputs["opinion_labels"])
    b_a = np.asarray(inputs["b_aspect"], np.float32)
    b_o = np.asarray(inputs["b_opinion"], np.float32)
    W_s = np.asarray(inputs["W_sent"], np.float32)
    b_s = np.asarray(inputs["b_sent"], np.float32)
    tr_a = np.asarray(inputs["trans_aspect"], np.float32)
    tr_o = np.asarray(inputs["trans_opinion"], np.float32)
    maskf = mask.astype(np.float32)

    # logits: [6, TOK] per core -> [B, S, 6]
    lt = np.stack([r["logits_t"] for r in results])          # [8, 6, 4096]
    lg = lt.reshape(NCORES, 6, BPC, S).transpose(0, 2, 3, 1).reshape(B, S, 6)
    aspect_logits = lg[..., 0:3] + b_a
    opinion_logits = lg[..., 3:6] + b_o

    # sentiment head from pooled sums; pooled_t[p, hc*BPC+b] = sum_s x[b, s, hc*128+p]
    plt = np.stack([r["pooled_t"] for r in results])         # [8, 128, 64]
    pooled = plt.reshape(NCORES, 128, NHC, BPC).transpose(0, 3, 2, 1).reshape(B, H)
    sentiment_logits = (pooled / np.float32(S)) @ W_s + b_s

    # CRF log-likelihoods
    emit = np.stack([r["emit"] for r in results])             # [8, 6, BPC]
    emit_a = float(emit[:, 0:3, :].sum())
    emit_o = float(emit[:, 3:6, :].sum())
    # bias contribution to emission scores (zero biases in practice)
    cnt_a = np.array([(maskf * (al == c)).sum() for c in range(T)])
    cnt_o = np.array([(maskf * (ol == c)).sum() for c in range(T)])
    emit_a += float(b_a @ cnt_a)
    emit_o += float(b_o @ cnt_o)

    pm = maskf[:, 1:] * maskf[:, :-1]
    trans_a = float((tr_a[al[:, :-1], al[:, 1:]] * pm).sum())
    trans_o = float((tr_o[ol[:, :-1], ol[:, 1:]] * pm).sum())

    cnt = float(maskf.sum())
    loss = -(emit_a + trans_a) / cnt - (emit_o + trans_o) / cnt

    return (
        aspect_logits.astype(np.float32),
        opinion_logits.astype(np.float32),
        sentiment_logits.astype(np.float32),
        np.float32(loss),
    )


def _run(inputs, trace=False):
    nc = _get_nc()
    in_maps = _prep_inputs(inputs)
    res = bass_utils.run_bass_kernel_spmd(
        nc, in_maps, core_ids=list(range(NCORES)), trace=trace
    )
    return _host_epilogue(inputs, res.results), res


def kernel(**inputs):
    out, _ = _run(inputs)
    return out
